# revision 1
# baseline (speedup 1.0000x reference)
"""Trainium2 Bass/Tile kernel for BasicCondConvBlock (E=1):
two CondConv1d(k=3,pad=1)+BN(eval)+LeakyReLU(0.1) blocks + MaxPool1d(2).

With a single expert, CondConv reduces to y_i = r_i * (conv(x_i, W) + b)
with a shared weight: conv runs on the TensorEngine as 3 shifted fp32r
matmuls accumulated in PSUM, and routing r_i + conv bias + BatchNorm fold
into one per-(sample,channel) affine:
    out = LeakyReLU( (r_i*s_c) * z + (r_i*b_c*s_c + be_c - rm_c*s_c) )
LeakyReLU is the hardware Prelu activation (alpha=0.1), fused with the
affine into a single ScalarE op.

Routing: the fc weight is host-replicated across 128 matmul output columns
so a single tiny matmul yields the logit already broadcast over all
partitions; sigmoid (ScalarE) then two tiny per-partition VectorE ops
produce the fused scale/bias columns — no PE outer products, and each
sample's chain is gated only on its own data (no batch barrier).

Block-1 epilogue: one ScalarE Prelu(affine) per tile writing fp32r with a
fused row-sum (feeds block-2 routing); 1-in-4 tiles drain on VectorE
instead to balance engine load.  Block-2 epilogue alternates two drain
structures per tile — (a) VectorE one-input 3D-AP max-pool from PSUM then
half-width ScalarE Prelu(affine) (exact: scale>0 keeps affine+Prelu
monotone), (b) full-width ScalarE Prelu(affine) then VectorE SBUF pool —
so the expensive PSUM reads split evenly across both engines.

DMA: bulk x/out on SWDGE (gpsimd; ~3x the HWDGE ring throughput at 1 MiB),
x one sample at a time so conv starts after ~0.5 MiB; w2 rides the ScalarE
HWDGE ring concurrently.

Sharding: pure data parallel over batch (32 samples -> 4 per core x 8).
"""

import numpy as np

N_CORES = 8
B, CIN, W = 32, 64, 2048
C1, C2 = 128, 256
BL = B // N_CORES  # samples per core
EPS = 1e-5
SLOPE = 0.1
WT = 512           # conv output tile width (one PSUM bank of fp32)
NT = W // WT       # 4
WO = W // 2        # pooled output width
HT = WT // 2

# wpk1 column layout: [0:384] w1 taps, [384:512] fcw1/W replicated x128,
# [512:523] misc consts (f32 bits): s1, t11, t21, s2a, s2b, t12a, t12b,
# t22a, t22b, fcb1, fcb2
W1C = 3 * C1
FC1 = W1C
CPK = FC1 + C1
NW1 = CPK + 11
W2C = 3 * C2
FC2 = W2C
NW2 = FC2 + C1

TRACE = False
LAST_RESULT = None

_built = None


def _build():
    global _built
    if _built is not None:
        return _built

    import concourse.bacc as bacc
    import concourse.mybir as mybir
    from concourse import tile
    from contextlib import ExitStack

    f32 = mybir.dt.float32
    f32r = mybir.dt.float32r
    Alu = mybir.AluOpType
    Act = mybir.ActivationFunctionType
    Ax = mybir.AxisListType

    nc = bacc.Bacc("TRN2", target_bir_lowering=False, debug=False)

    xd = nc.declare_dram_parameter("x", [BL, CIN, W + 2], f32r, isOutput=False)
    w1d = nc.declare_dram_parameter("wpk1", [2 * CIN, NW1], f32r, isOutput=False)
    w2d = nc.declare_dram_parameter("wpk2", [C1, NW2], f32r, isOutput=False)
    od = nc.declare_dram_parameter("out", [BL, C2, WO], f32, isOutput=True)
    x_ap, w1_ap, w2_ap, o_ap = xd.ap(), w1d.ap(), w2d.ap(), od.ap()

    def conv_taps(zp, off, lhsT, src, c0):
        """Accumulate the 3-tap conv for output cols [c0, c0+WT) of one
        128-wide output-channel chunk into zp[:, off:off+WT].  lhsT(k) ->
        [K,128] stationary AP; src -> [K, W+2] zero-padded input AP (data at
        cols 1..W).  All taps full width: fp32r matmuls need even N and
        8B-aligned PSUM offsets."""
        for k in range(3):
            nc.tensor.matmul(zp[:, off : off + WT], lhsT(k),
                             src[:, c0 + k : c0 + k + WT],
                             start=(k == 0), stop=(k == 2))

    W2T = 2 * WT  # PSUM tiles span two banks; drains amortize per-op overhead

    with tile.TileContext(nc) as tc:
        with ExitStack() as ctx:
            consts = ctx.enter_context(tc.tile_pool(name="consts", bufs=1))
            xpool = ctx.enter_context(tc.tile_pool(name="xp", bufs=2))
            y1pool = ctx.enter_context(tc.tile_pool(name="y1p", bufs=BL))
            pmp = ctx.enter_context(tc.tile_pool(name="pmp", bufs=8))
            outp = ctx.enter_context(tc.tile_pool(name="outp", bufs=3))
            small = ctx.enter_context(tc.tile_pool(name="small", bufs=1))
            psum = ctx.enter_context(tc.tile_pool(name="psum", bufs=3, space="PSUM"))
            psmall = ctx.enter_context(tc.tile_pool(name="psm", bufs=2, space="PSUM"))

            # --- input DMAs
            w1s = consts.tile([2 * CIN, NW1], f32r)
            nc.gpsimd.dma_start(out=w1s[:], in_=w1_ap[:])
            w2s = consts.tile([C1, NW2], f32r)
            nc.scalar.dma_start(out=w2s[:], in_=w2_ap[:])

            xts = [
                xpool.tile([2 * CIN, W + 2], f32r, tag="xt", name=f"xt{i}")
                for i in range(BL // 2)
            ]
            for i in range(BL // 2):
                nc.gpsimd.dma_start(
                    out=xts[i][:],
                    in_=x_ap[2 * i : 2 * i + 2].rearrange("s c w -> (s c) w"),
                )

            def xv(s):
                return xts[s // 2][(s % 2) * CIN : (s % 2 + 1) * CIN, :]

            def cpk(j):
                return w1s[:, CPK + j : CPK + j + 1].bitcast(f32)

            # per-sample column sums of x -> routing 1.  GpSimd (otherwise
            # idle) folds the 2048 columns in half, DVE finishes the reduce;
            # each sample's scan is gated only on its own half of the x tile.
            m1 = small.tile([2 * CIN, BL // 2], f32)
            H2 = W // 2
            for s in range(BL):
                rows = slice((s % 2) * CIN, (s % 2 + 1) * CIN)
                gscr = small.tile([2 * CIN, H2], f32, tag="gscr", name=f"gscr{s}")
                nc.gpsimd.tensor_add(
                    gscr[rows, :],
                    xts[s // 2][rows, 1 : 1 + H2].bitcast(f32),
                    xts[s // 2][rows, 1 + H2 : 1 + W].bitcast(f32),
                )
                nc.vector.reduce_sum(
                    m1[rows, s // 2 : s // 2 + 1], gscr[rows, :], axis=Ax.X
                )

            # routing-1 chain per sample: one matmul against the replicated fc
            # weight yields the logit broadcast over all 128 partitions, then
            # sigmoid and two tiny VectorE ops build the scale/bias columns.
            rbc1 = small.tile([C1, BL], f32)
            sc1 = small.tile([C1, BL], f32)
            bi1 = small.tile([C1, BL], f32)
            for s in range(BL):
                rows = slice((s % 2) * CIN, (s % 2 + 1) * CIN)
                lgb = psmall.tile([C1, 1], f32, tag="sm", name=f"lg1{s}")
                nc.tensor.matmul(
                    lgb[:],
                    w1s[rows, FC1 : FC1 + C1].bitcast(f32),
                    m1[rows, s // 2 : s // 2 + 1],
                    start=True, stop=True,
                )
                nc.scalar.activation(
                    rbc1[:, s : s + 1], lgb[:], Act.Sigmoid,
                    bias=cpk(9), scale=1.0,
                )
                nc.vector.tensor_scalar(
                    sc1[:, s : s + 1], cpk(0), rbc1[:, s : s + 1], None, Alu.mult
                )
                nc.vector.scalar_tensor_tensor(
                    bi1[:, s : s + 1], cpk(1), rbc1[:, s : s + 1], cpk(2),
                    Alu.mult, Alu.add,
                )

            # ---- block 1 + per-sample routing-2 chain
            s1acc = small.tile([C1, BL * 2], f32)
            ssum = small.tile([C1, BL], f32)
            rbc2 = small.tile([C1, BL], f32)
            sc2 = small.tile([C1, 2 * BL], f32)
            bi2 = small.tile([C1, 2 * BL], f32)
            y1s = []
            for s in range(BL):
                y1 = y1pool.tile([C1, W + 2], f32r, tag="y1")
                # zero the two padding columns with an fp32r-writing DVE op
                # (memset cannot emit fp32r); inputs only feed a *0.0
                nc.vector.scalar_tensor_tensor(
                    y1[:, 0 : W + 2 : W + 1],
                    w1s[:, CPK : CPK + 2].bitcast(f32), 0.0,
                    w1s[:, CPK : CPK + 2].bitcast(f32), Alu.mult, Alu.mult,
                )
                half = s % 2
                w1v = lambda k, h=half: w1s[
                    h * CIN : (h + 1) * CIN, k * C1 : (k + 1) * C1
                ]
                for d in range(2):
                    zp = psum.tile([C1, W2T], f32, tag="zp")
                    conv_taps(zp, 0, w1v, xv(s), W2T * d)
                    conv_taps(zp, WT, w1v, xv(s), W2T * d + WT)
                    acc = s1acc[:, 2 * s + d : 2 * s + d + 1]
                    dst = y1[:, 1 + W2T * d : 1 + W2T * (d + 1)]
                    if not (half == 1 and d == 1):
                        # ScalarE drain: fused Prelu(affine) + row-sum
                        nc.scalar.activation(
                            dst, zp[:], Act.Prelu,
                            bias=bi1[:, s : s + 1], scale=sc1[:, s : s + 1],
                            alpha=SLOPE, accum_out=acc,
                        )
                    else:
                        # VectorE drain for 1-in-4 doubles: balances load
                        ytmp = pmp.tile([C1, W2T], f32, tag="ytmp")
                        nc.vector.tensor_scalar(
                            ytmp[:], zp[:],
                            sc1[:, s : s + 1], bi1[:, s : s + 1],
                            Alu.mult, Alu.add,
                        )
                        nc.vector.scalar_tensor_tensor(
                            dst, ytmp[:], SLOPE, ytmp[:], Alu.mult, Alu.max,
                            accum_out=acc,
                        )
                y1s.append(y1)

                # routing-2 chain, gated only on this sample's block-1 drains
                nc.vector.reduce_sum(
                    ssum[:, s : s + 1], s1acc[:, 2 * s : 2 * (s + 1)], axis=Ax.X
                )
                lgb2 = psmall.tile([C1, 1], f32, tag="sm", name=f"lg2{s}")
                nc.tensor.matmul(
                    lgb2[:],
                    w2s[:, FC2 : FC2 + C1].bitcast(f32),
                    ssum[:, s : s + 1],
                    start=True, stop=True,
                )
                nc.scalar.activation(
                    rbc2[:, s : s + 1], lgb2[:], Act.Sigmoid,
                    bias=cpk(10), scale=1.0,
                )
                for c in range(2):
                    nc.vector.tensor_scalar(
                        sc2[:, c * BL + s : c * BL + s + 1], cpk(3 + c),
                        rbc2[:, s : s + 1], None, Alu.mult,
                    )
                    nc.vector.scalar_tensor_tensor(
                        bi2[:, c * BL + s : c * BL + s + 1], cpk(5 + c),
                        rbc2[:, s : s + 1], cpk(7 + c), Alu.mult, Alu.add,
                    )

            # ---- block 2: conv(128->256); alternating PSUM-drain structure
            for s in range(BL):
                for c in range(2):
                    ot = outp.tile([C1, WO], f32, tag="ot")
                    w2v = lambda k, cc=c: w2s[:, k * C2 + C1 * cc : k * C2 + C1 * cc + C1]
                    sc_col = sc2[:, c * BL + s : c * BL + s + 1]
                    bi_col = bi2[:, c * BL + s : c * BL + s + 1]
                    for d in range(2):
                        zp2 = psum.tile([C1, W2T], f32, tag="zp")
                        conv_taps(zp2, 0, w2v, y1s[s], W2T * d)
                        conv_taps(zp2, WT, w2v, y1s[s], W2T * d + WT)
                        if d == 0:
                            # VectorE drains PSUM: one-input 3D-AP max-pool,
                            # then ScalarE Prelu(affine) at half width (exact
                            # since scale>0 keeps affine+Prelu monotone)
                            pm = pmp.tile([C1, WT], f32, tag="pm")
                            nc.vector.tensor_reduce(
                                pm[:], zp2[:].rearrange("p (a b) -> p a b", b=2),
                                axis=Ax.X, op=Alu.max,
                            )
                            nc.scalar.activation(
                                ot[:, WT * d : WT * (d + 1)], pm[:], Act.Prelu,
                                bias=bi_col, scale=sc_col, alpha=SLOPE,
                            )
                        else:
                            # ScalarE drains PSUM: full-width Prelu(affine),
                            # then VectorE pools from SBUF.  Alternating the
                            # two structures splits the expensive PSUM reads
                            # evenly across both engines.
                            yw = pmp.tile([C1, W2T], f32, tag="yw")
                            nc.scalar.activation(
                                yw[:], zp2[:], Act.Prelu,
                                bias=bi_col, scale=sc_col, alpha=SLOPE,
                            )
                            nc.vector.tensor_tensor(
                                ot[:, WT * d : WT * (d + 1)],
                                yw[:, 0:W2T:2], yw[:, 1:W2T:2], Alu.max,
                            )
                    nc.gpsimd.dma_start(
                        out=o_ap[s, C1 * c : C1 * (c + 1), :], in_=ot[:]
                    )

    nc.compile()
    _built = nc
    return nc


def _pack_inputs(x, w1, b1, fcw1, fcb1, g1, be1, rm1, rv1,
                 w2, b2, fcw2, fcb2, g2, be2, rm2, rv2):
    f = np.float32
    s1 = (g1 / np.sqrt(rv1 + EPS)).astype(f)
    s2 = (g2 / np.sqrt(rv2 + EPS)).astype(f)
    t11, t21 = (b1[0] * s1).astype(f), (be1 - rm1 * s1).astype(f)
    t12, t22 = (b2[0] * s2).astype(f), (be2 - rm2 * s2).astype(f)

    w1t = w1[0].transpose(1, 2, 0).reshape(CIN, 3 * C1).astype(f)
    w2t = w2[0].transpose(1, 2, 0).reshape(C1, 3 * C2).astype(f)

    wpk1 = np.zeros((2 * CIN, NW1), f)
    for h in range(2):
        r = slice(h * CIN, (h + 1) * CIN)
        wpk1[r, 0:W1C] = w1t
        wpk1[r, FC1:FC1 + C1] = (fcw1[0] / W)[:, None]
    cols = [s1, t11, t21, s2[:C1], s2[C1:], t12[:C1], t12[C1:],
            t22[:C1], t22[C1:], np.full(C1, fcb1[0], f), np.full(C1, fcb2[0], f)]
    for j, col in enumerate(cols):
        wpk1[:, CPK + j] = col

    wpk2 = np.zeros((C1, NW2), f)
    wpk2[:, 0:W2C] = w2t
    wpk2[:, FC2:FC2 + C1] = (fcw2[0] / W)[:, None]

    com = {"wpk1": wpk1, "wpk2": wpk2}
    xp = np.zeros((B, CIN, W + 2), f)
    xp[:, :, 1 : W + 1] = x
    return [
        {**com, "x": np.ascontiguousarray(xp[i * BL : (i + 1) * BL])}
        for i in range(N_CORES)
    ]


def _enable_trace():
    """Register the NTFF profile hook (absent antenv.axon_hooks on this image)
    and stub out the S3 artifact upload so trace=True works locally."""
    import sys
    import types

    import concourse.bass_utils as bu

    bu.upload_artifacts = lambda tmpdir: tmpdir
    if "antenv.axon_hooks" not in sys.modules:
        import antenv
        from trn_agent_boot.trn_boot import _ntff_profile_via_ctypes

        hooks = types.ModuleType("antenv.axon_hooks")
        _store = {"hook": _ntff_profile_via_ctypes("/opt/axon/libaxon_pjrt.so")}
        hooks.set_axon_ntff_profile_hook = lambda h: _store.__setitem__("hook", h)
        hooks.get_axon_ntff_profile_hook = lambda: _store["hook"]
        sys.modules["antenv.axon_hooks"] = hooks
        antenv.axon_hooks = hooks


def kernel(**inputs):
    global LAST_RESULT
    from concourse.bass_utils import run_bass_kernel_spmd

    if TRACE:
        _enable_trace()
    nc = _build()
    in_maps = _pack_inputs(**inputs)
    res = run_bass_kernel_spmd(nc, in_maps, list(range(N_CORES)), trace=TRACE)
    LAST_RESULT = res
    return np.concatenate([r["out"] for r in res.results], axis=0)

